# revision 2
# baseline (speedup 1.0000x reference)
"""GST-LSTM cell (graph-conv LSTM) on 8 Trainium2 NeuronCores.

Computation (reference):
    g  = adj_matrix @ Ht_1                       # (N, H)  -- dominant cost
    i  = sigmoid(ht @ Wxi.T + bxi + g @ Whi.T + bhi)
    f  = sigmoid(ht @ Wxf.T + bxf + g @ Whf.T + bhf)
    o  = sigmoid(ht @ Wxo.T + bxo + g @ Who.T + bho)
    u  = tanh   (ht @ Wxc.T + bxc + g @ Whc.T + bhc)
    Ct = f * Ct_1 + it * u
    Ht = o * tanh(Ct)

Sharding: node dim (rows of adj, ht, Ct_1; output rows) split across the
8 cores; Ht_1 replicated. No collectives needed.

Device layout: everything is computed feature-major ([64, nodes] tiles)
so the PE contraction dim lands on partitions without on-device
transposes:
  - adj is transposed, shifted by -0.5, cast to fp16 and PERMUTED on the
    host into stripe-contiguous order: each [128 k-rows, KTB*512] stripe
    is one contiguous 4 MiB region of DRAM. Each stripe is FETCHED in
    four 1 MiB column-chunk DMAs alternating between the two HWDGE rings
    (SP via nc.sync, ACT via nc.scalar): PE consumes 8 k-tiles per chunk
    as it lands, so PE idle gaps stay under the ~3.4 us HAM re-throttle
    window (a whole-stripe fetch left PE idle ~8 us per stripe and its
    matmuls restarted at the 1.2 GHz cold clock), and the kernel's first
    matmul starts ~3 us in while the last stripe drains incrementally.
  - Ht_1 enters as an fp16 hi/lo pair packed side by side per k-tile
    ([128, kt*128]): one [128,128] stationary computes both products in a
    single matmul pass (PSUM rows 0:64 accumulate A@H_hi, rows 64:128
    A@H_lo). It is loaded in 8 chunks on the ACT ring behind the two
    small consts (wxt, xt) the first matmuls need.
  - the eight 64x64 Linears run feature-major; the x-side (ht) matmuls
    are fp16 and hoisted to m-block start so only the fp32 h-side matmul
    + activation + gating sit on the tail. Biases enter via the ACT
    engine's per-partition bias operand.
  - the gate tail runs in four 128-col quarter slices (deeper
    PE->ACT->DVE pipelining); quarters land in fp16 staging tiles so
    each m-block issues two batched output stores. Mid-stream stores are
    deferred into the next m-block's stripe stream (they never stall the
    HWDGE sequencers); the last m-block stores directly on the idle
    rings. Outputs are fp16 (error contribution ~5e-4, halves store
    traffic); the host casts back to fp32.

fp16 for the adj @ Ht_1 product keeps end-to-end relative error at the
~1.2e-2 level (fp32 PSUM accumulation) while halving HBM traffic of the
1 GiB adjacency stream, which is what the memory-bound regime rewards.
"""

import numpy as np

N = 16384
D = 64
N_CORES = 8
ROWS = N // N_CORES          # 2048 nodes per core
MBW = 512                    # m-block width (PE moving free dim / PSUM bank)
MB = ROWS // MBW             # 4 m-blocks per core
KT = N // 128                # 128 k-tiles of 128 contraction rows
KTB = 32                     # k-tiles per stripe (4 MiB stripe-contiguous)
GD = KT // KTB               # stripes per m-block
HHC = 8                      # hh load chunks

_GATE_FUNCS = ("Sigmoid", "Sigmoid", "Sigmoid", "Tanh")  # i, f, o, u


def _split_excess_waits(nc, max_waits=1):
    """Split >max_waits sem waits off instructions onto preceding NOPs.

    The walrus build here rejects instructions carrying more than a
    couple of sync waits ("Too many sync wait commands" from
    setupSyncWait during codegen). Tile's wait assignment doesn't know
    that limit; an NX engine executes its stream in order, so moving
    the excess waits onto same-engine NOPs directly before the
    instruction preserves ordering semantics with a legal encoding.
    """
    from concourse import mybir

    fn = nc.m.functions[0]
    for bb in fn.blocks:
        out = []
        for inst in bb.instructions:
            si = getattr(inst, "sync_info", None)
            if si is not None and si.on_wait and len(si.on_wait) > max_waits:
                waits = list(si.on_wait)
                spill, keep = waits[:-max_waits], waits[-max_waits:]
                for i in range(0, len(spill), max_waits):
                    nop = mybir.InstNoOp(
                        name=nc.get_next_instruction_name(),
                        sync_info=mybir.SyncInfo(
                            on_wait=spill[i:i + max_waits], on_update=[]
                        ),
                        bass_nofuse=True,
                        engine=inst.engine,
                    )
                    out.append(nop)
                si.on_wait = keep
            out.append(inst)
        bb.instructions[:] = out


def build(n=N, rows=ROWS, mbw=MBW, ktb=KTB, repeat=1, adj_bufs=3,
          split_waits=True, unroll=1, tail_splits=4, chunk_edges=4,
          out_f16=True):
    """Build the per-core Bass program. All cores run the same program."""
    import concourse.bass as bass
    import concourse.mybir as mybir
    from concourse import tile

    dt = mybir.dt
    act = mybir.ActivationFunctionType
    f16, f32 = dt.float16, dt.float32

    kt = n // 128
    mb = rows // mbw
    gd = kt // ktb
    hkc = kt // HHC              # k-tiles per hh chunk

    nc = bass.Bass()
    # stripe-contiguous: row block (mbi*gd + g)*128 .. +128 is one stripe
    adjt = nc.declare_dram_parameter("adjt", [mb * gd * 128, ktb * mbw], f16,
                                     isOutput=False)
    # hh packs [H_hi | H_lo] fp16 side by side per k-tile: one [128,128]
    # stationary computes both products in a single matmul pass.
    hh = nc.declare_dram_parameter("hh", [128, kt * 2 * D], f16, isOutput=False)
    xt = nc.declare_dram_parameter("xt", [D, rows], f16, isOutput=False)
    ct = nc.declare_dram_parameter("ct", [D, rows], f32, isOutput=False)
    wxt = nc.declare_dram_parameter("wxt", [D, 4 * D], f16, isOutput=False)
    wht = nc.declare_dram_parameter("wht", [D, 4 * D], f32, isOutput=False)
    bias = nc.declare_dram_parameter("bias", [D, 4], f32, isOutput=False)
    fo = f16 if out_f16 else f32
    ht_out = nc.declare_dram_parameter("ht_out", [D, rows], fo, isOutput=True)
    ct_out = nc.declare_dram_parameter("ct_out", [D, rows], fo, isOutput=True)

    with tile.TileContext(nc) as tc:
        with (
            tc.tile_pool(name="const", bufs=1) as cst,
            tc.tile_pool(name="adj", bufs=adj_bufs) as apool,
            tc.tile_pool(name="b64", bufs=3) as b64,
            tc.tile_pool(name="stg", bufs=2) as stg,
            tc.tile_pool(name="gpsum", bufs=2, space="PSUM") as gpsum,
            tc.tile_pool(name="gatepsum", bufs=1, space="PSUM") as gatepsum,
        ):
            dma_engs = [nc.sync, nc.scalar]

            # SP ring: the two tiny consts PE needs first, then stripe 0.
            # ACT ring: hh chunks, then the tail-only consts.
            wxt_sb = cst.tile([D, 4 * D], f16)
            nc.sync.dma_start(wxt_sb[:], wxt[:])
            xt_sb = cst.tile([D, rows], f16)
            nc.sync.dma_start(xt_sb[:], xt[:])
            hh_sb = []
            for c in range(HHC):
                t = cst.tile([128, hkc * 2 * D], f16, tag=f"hh{c}")
                nc.scalar.dma_start(
                    t[:], hh[:, c * hkc * 2 * D:(c + 1) * hkc * 2 * D]
                )
                hh_sb.append(t)
            bias_sb = cst.tile([D, 4], f32)
            nc.scalar.dma_start(bias_sb[:], bias[:])
            wht_sb = cst.tile([D, 4 * D], f32)
            nc.scalar.dma_start(wht_sb[:], wht[:])
            ct_sb = cst.tile([D, rows], f32)
            nc.scalar.dma_start(ct_sb[:], ct[:])

            def body(_iv=None):
                # batched output stores of m-block i are deferred into
                # m-block i+1's stripe stream: by then they are computed,
                # so they never stall the HWDGE sequencers.
                pending = []

                for mbi in range(mb):
                    mbs = slice(mbi * mbw, (mbi + 1) * mbw)

                    # x-side gate matmuls: independent of the adjacency
                    # stream; fill PE early, leave only h-side on the tail.
                    pgs = []
                    for gi in range(4):
                        pg = gatepsum.tile([D, mbw], f32, tag=f"pg{gi}")
                        nc.tensor.matmul(
                            pg[:],
                            wxt_sb[:, gi * D:(gi + 1) * D],
                            xt_sb[:, mbs],
                            start=True,
                            stop=False,
                        )
                        pgs.append(pg)

                    gps = gpsum.tile([128, mbw], f32, tag="gps")
                    for g in range(gd):
                        first = (mbi == 0 and g == 0)
                        stripe = apool.tile([128, ktb * mbw], f16, tag="stripe")
                        rb = (mbi * gd + g) * 128
                        # every stripe is fetched in column chunks: PE gets
                        # work every ~chunk-DMA-time instead of idling a
                        # whole stripe DMA (keeps HAM from re-throttling),
                        # and boundary stripes start/finish incrementally.
                        cw = ktb * mbw // chunk_edges
                        ca = ktb // chunk_edges       # k-tiles per chunk
                        for ci in range(chunk_edges):
                            eng = dma_engs[0] if first else dma_engs[ci % 2]
                            eng.dma_start(
                                stripe[:, ci * cw:(ci + 1) * cw],
                                adjt[rb:rb + 128, ci * cw:(ci + 1) * cw],
                            )
                            if g == 1 and ci == 0 and pending:
                                for pi, (dst, tsb) in enumerate(pending):
                                    dma_engs[pi % 2].dma_start(dst, tsb[:])
                                pending.clear()
                            for a in range(ci * ca, (ci + 1) * ca):
                                kti = g * ktb + a
                                hc, ho = kti // hkc, kti % hkc
                                nc.tensor.matmul(
                                    gps[:],
                                    hh_sb[hc][:, ho * 2 * D:(ho + 1) * 2 * D],
                                    stripe[:, a * mbw:(a + 1) * mbw],
                                    start=(kti == 0),
                                    stop=(kti == kt - 1),
                                )
                    # tail in quarter-width slices so PE/ACT/DVE pipeline;
                    # quarters land in per-m-block staging tiles so each
                    # m-block issues just two batched output stores.
                    cts = stg.tile([D, mbw], fo, tag="cts")
                    hts = stg.tile([D, mbw], fo, tag="hts")
                    hw = mbw // tail_splits
                    for h in range(tail_splits):
                        hs = slice(h * hw, (h + 1) * hw)       # within m-block
                        hg = slice(mbi * mbw + h * hw, mbi * mbw + (h + 1) * hw)
                        gtb = b64.tile([D, hw], f32, tag=f"gtb{h}")
                        nc.vector.tensor_copy(gtb[:], gps[0:D, hs])
                        nc.vector.tensor_add(gtb[:], gtb[:], gps[D:2 * D, hs])
                        gates = []
                        for gi, fname in enumerate(_GATE_FUNCS):
                            pg = pgs[gi]
                            nc.tensor.matmul(
                                pg[:, hs],
                                wht_sb[:, gi * D:(gi + 1) * D],
                                gtb[:],
                                start=False,
                                stop=(h == tail_splits - 1),
                                skip_group_check=True,
                            )
                            gate_sb = b64.tile([D, hw], f32, tag=f"gate{gi}{h}")
                            nc.scalar.activation(
                                gate_sb[:],
                                pg[:, hs],
                                getattr(act, fname),
                                bias=bias_sb[:, gi:gi + 1],
                            )
                            gates.append(gate_sb)
                        it_, ft_, ot_, ut_ = gates

                        t1 = b64.tile([D, hw], f32, tag=f"t1{h}")
                        nc.vector.tensor_mul(t1[:], ft_[:], ct_sb[:, hg])
                        t2 = b64.tile([D, hw], f32, tag=f"t2{h}")
                        nc.vector.tensor_mul(t2[:], it_[:], ut_[:])
                        nc.vector.tensor_add(cts[:, hs], t1[:], t2[:])
                        tct = b64.tile([D, hw], f32, tag=f"tct{h}")
                        nc.scalar.activation(tct[:], cts[:, hs], act.Tanh)
                        nc.vector.tensor_mul(hts[:, hs], ot_[:], tct[:])
                    og = slice(mbi * mbw, (mbi + 1) * mbw)
                    if mbi == mb - 1:
                        # HWDGE rings are idle at the end: issue directly
                        nc.sync.dma_start(ct_out[:, og], cts[:])
                        nc.scalar.dma_start(ht_out[:, og], hts[:])
                    else:
                        pending.append((ct_out[:, og], cts))
                        pending.append((ht_out[:, og], hts))

            if repeat == 1:
                for _ in range(unroll):
                    body()
            else:
                # unroll>1 amortizes the per-trip InstAllEngineBarrier that
                # For_i inserts in its semaphore-reset block
                with tc.For_i(0, repeat, 1) as _i:
                    for _ in range(unroll):
                        body(_i)

    if split_waits:
        _split_excess_waits(nc)
    return nc


def make_in_maps(inputs, n=N, n_cores=N_CORES):
    """Host-side sharding + relayout. Returns per-core input dicts."""
    rows = n // n_cores
    kt = n // 128
    mb = rows // MBW
    gd = kt // KTB
    adj = np.asarray(inputs["adj_matrix"], dtype=np.float32)
    H = np.asarray(inputs["Ht_1"], dtype=np.float32)
    ht = np.asarray(inputs["ht"], dtype=np.float32)
    Ct_1 = np.asarray(inputs["Ct_1"], dtype=np.float32)

    # H = hi + lo to ~2^-22: the hi/lo fp16 pair is packed side by side
    # per k-tile ([128, kt*128]) so one matmul computes both products.
    Hh32 = H.astype(np.float16).astype(np.float32)
    packed = np.empty((128, kt, 2 * D), dtype=np.float16)
    packed[:, :, :D] = Hh32.reshape(kt, 128, D).transpose(1, 0, 2)
    packed[:, :, D:] = (H - Hh32).reshape(kt, 128, D).transpose(1, 0, 2)
    hh = np.ascontiguousarray(packed.reshape(128, kt * 2 * D))

    gate_w = ("Wxi", "Wxf", "Wxo", "Wxc")
    gate_h = ("Whi", "Whf", "Who", "Whc")
    wxt = np.concatenate(
        [np.asarray(inputs[g + "_w"], np.float32).T for g in gate_w], axis=1
    ).astype(np.float16)
    wht = np.concatenate(
        [np.asarray(inputs[g + "_w"], np.float32).T for g in gate_h], axis=1
    )
    # adj is shifted by -0.5 before the fp16 cast (halves quantization
    # error for uniform(0,1) entries). g = (adj-0.5)@H + 0.5*colsum(H)
    # broadcast over rows; the second term passes through the h-side
    # Linear as a per-feature constant, folded into the gate bias here.
    colsum = H.astype(np.float64).sum(axis=0)
    bias = np.stack(
        [
            np.asarray(inputs[gx + "_b"], np.float64)
            + np.asarray(inputs[gh + "_b"], np.float64)
            + 0.5 * (np.asarray(inputs[gh + "_w"], np.float64) @ colsum)
            for gx, gh in zip(gate_w, gate_h)
        ],
        axis=1,
    ).astype(np.float32)
    wxt = np.ascontiguousarray(wxt)
    wht = np.ascontiguousarray(wht)
    bias = np.ascontiguousarray(bias)

    in_maps = []
    for c in range(n_cores):
        rs = slice(c * rows, (c + 1) * rows)
        adjt_c = np.ascontiguousarray(adj[rs].T)
        adjt_c -= np.float32(0.5)
        a16 = adjt_c.astype(np.float16)          # [n, rows]
        # stripe-contiguous permute: [(mb gd) 128, ktb*mbw] where the row
        # block (mbi*gd+g)*128 holds k-rows (g*ktb .. )*128 interleaved as
        # [p, a, mj] -> flat [128, ktb*mbw] for m-cols mbi*mbw..+mbw.
        a5 = a16.reshape(gd, KTB, 128, mb, MBW).transpose(3, 0, 2, 1, 4)
        adjt_s = np.ascontiguousarray(a5.reshape(mb * gd * 128, KTB * MBW))
        in_maps.append(
            {
                "adjt": adjt_s,
                "hh": hh,
                "xt": np.ascontiguousarray(ht[rs].T).astype(np.float16),
                "ct": np.ascontiguousarray(Ct_1[rs].T),
                "wxt": wxt,
                "wht": wht,
                "bias": bias,
            }
        )
    return in_maps


def gather(results):
    Ht = np.concatenate(
        [np.asarray(r["ht_out"], np.float32).T for r in results], axis=0
    )
    Ct = np.concatenate(
        [np.asarray(r["ct_out"], np.float32).T for r in results], axis=0
    )
    return np.ascontiguousarray(Ht), np.ascontiguousarray(Ct)


_PROGRAM_CACHE = {}


def kernel(**inputs):
    from concourse.bass_utils import run_bass_kernel_spmd

    if "nc" not in _PROGRAM_CACHE:
        _PROGRAM_CACHE["nc"] = build()
    nc = _PROGRAM_CACHE["nc"]
    in_maps = make_in_maps(inputs)
    res = run_bass_kernel_spmd(nc, in_maps, list(range(N_CORES)))
    return gather(res.results)


# revision 6
# speedup vs baseline: 1.1821x; 1.1821x over previous
"""GST-LSTM cell (graph-conv LSTM) on 8 Trainium2 NeuronCores.

Computation (reference):
    g  = adj_matrix @ Ht_1                       # (N, H)  -- dominant cost
    i  = sigmoid(ht @ Wxi.T + bxi + g @ Whi.T + bhi)
    f  = sigmoid(ht @ Wxf.T + bxf + g @ Whf.T + bhf)
    o  = sigmoid(ht @ Wxo.T + bxo + g @ Who.T + bho)
    u  = tanh   (ht @ Wxc.T + bxc + g @ Whc.T + bhc)
    Ct = f * Ct_1 + it * u
    Ht = o * tanh(Ct)

Sharding: node dim (rows of adj, ht, Ct_1; output rows) split across the
8 cores; Ht_1 replicated. No collectives needed.

Device layout: everything is computed feature-major ([64, nodes] tiles)
so the PE contraction dim lands on partitions without on-device
transposes:
  - adj is transposed, shifted by -0.5, cast to fp16 and PERMUTED on the
    host into stripe-contiguous order: each [128 k-rows, KTB*512] stripe
    is one contiguous 4 MiB region of DRAM. Each stripe is FETCHED in
    four 1 MiB column-chunk DMAs alternating between the two HWDGE rings
    (SP via nc.sync, ACT via nc.scalar): PE consumes 8 k-tiles per chunk
    as it lands, so PE idle gaps stay under the ~3.4 us HAM re-throttle
    window (a whole-stripe fetch left PE idle ~8 us per stripe and its
    matmuls restarted at the 1.2 GHz cold clock), and the kernel's first
    matmul starts ~3 us in while the last stripe drains incrementally.
  - Ht_1 enters as an fp16 hi/lo pair packed side by side per k-tile
    ([128, kt*128]): one [128,128] stationary computes both products in a
    single matmul pass (PSUM rows 0:64 accumulate A@H_hi, rows 64:128
    A@H_lo). It is loaded in 8 chunks on the ACT ring behind the two
    small consts (wxt, xt) the first matmuls need.
  - the eight 64x64 Linears run feature-major; the x-side (ht) matmuls
    are fp16 and hoisted to m-block start so only the fp32 h-side matmul
    + activation + gating sit on the tail. Biases enter via the ACT
    engine's per-partition bias operand.
  - the gate tail runs in four 128-col quarter slices (deeper
    PE->ACT->DVE pipelining); quarters land in fp16 staging tiles so
    each m-block issues two batched output stores. Mid-stream stores are
    deferred into the next m-block's stripe stream (they never stall the
    HWDGE sequencers); the last m-block stores directly on the idle
    rings. Outputs are fp16 (error contribution ~5e-4, halves store
    traffic); the host casts back to fp32.

fp16 for the adj @ Ht_1 product keeps end-to-end relative error at the
~1.2e-2 level (fp32 PSUM accumulation) while halving HBM traffic of the
1 GiB adjacency stream, which is what the memory-bound regime rewards.
"""

import numpy as np

N = 16384
D = 64
N_CORES = 8
ROWS = N // N_CORES          # 2048 nodes per core
MBW = 512                    # m-block width (PE moving free dim / PSUM bank)
MB = ROWS // MBW             # 4 m-blocks per core
KT = N // 128                # 128 k-tiles of 128 contraction rows
KTB = 32                     # k-tiles per stripe (4 MiB stripe-contiguous)
GD = KT // KTB               # stripes per m-block
HHC = 8                      # hh load chunks

_GATE_FUNCS = ("Sigmoid", "Sigmoid", "Sigmoid", "Tanh")  # i, f, o, u


def _split_excess_waits(nc, max_waits=1):
    """Split >max_waits sem waits off instructions onto preceding NOPs.

    The walrus build here rejects instructions carrying more than a
    couple of sync waits ("Too many sync wait commands" from
    setupSyncWait during codegen). Tile's wait assignment doesn't know
    that limit; an NX engine executes its stream in order, so moving
    the excess waits onto same-engine NOPs directly before the
    instruction preserves ordering semantics with a legal encoding.
    """
    from concourse import mybir

    fn = nc.m.functions[0]
    for bb in fn.blocks:
        out = []
        for inst in bb.instructions:
            si = getattr(inst, "sync_info", None)
            if si is not None and si.on_wait and len(si.on_wait) > max_waits:
                waits = list(si.on_wait)
                spill, keep = waits[:-max_waits], waits[-max_waits:]
                for i in range(0, len(spill), max_waits):
                    nop = mybir.InstNoOp(
                        name=nc.get_next_instruction_name(),
                        sync_info=mybir.SyncInfo(
                            on_wait=spill[i:i + max_waits], on_update=[]
                        ),
                        bass_nofuse=True,
                        engine=inst.engine,
                    )
                    out.append(nop)
                si.on_wait = keep
            out.append(inst)
        bb.instructions[:] = out


def build(n=N, rows=ROWS, mbw=MBW, ktb=KTB, repeat=1, adj_bufs=3,
          split_waits=True, unroll=1, tail_splits=4, chunk_edges=4,
          out_f16=True, b64_bufs=3, stg_bufs=2):
    """Build the per-core Bass program. All cores run the same program."""
    import concourse.bass as bass
    import concourse.mybir as mybir
    from concourse import tile

    dt = mybir.dt
    act = mybir.ActivationFunctionType
    f16, f32 = dt.float16, dt.float32

    kt = n // 128
    mb = rows // mbw
    gd = kt // ktb
    hkc = kt // HHC              # k-tiles per hh chunk

    nc = bass.Bass()
    # stripe-contiguous: row block (mbi*gd + g)*128 .. +128 is one stripe
    adjt = nc.declare_dram_parameter("adjt", [mb * gd * 128, ktb * mbw], f16,
                                     isOutput=False)
    # hh packs [H_hi | H_lo] fp16 side by side per k-tile: one [128,128]
    # stationary computes both products in a single matmul pass.
    hh = nc.declare_dram_parameter("hh", [128, kt * 2 * D], f16, isOutput=False)
    xt = nc.declare_dram_parameter("xt", [D, rows], f16, isOutput=False)
    ct = nc.declare_dram_parameter("ct", [D, rows], f32, isOutput=False)
    wxt = nc.declare_dram_parameter("wxt", [D, 4 * D], f16, isOutput=False)
    wht = nc.declare_dram_parameter("wht", [D, 4 * D], f32, isOutput=False)
    bias = nc.declare_dram_parameter("bias", [D, 4], f32, isOutput=False)
    fo = f16 if out_f16 else f32
    ht_out = nc.declare_dram_parameter("ht_out", [D, rows], fo, isOutput=True)
    ct_out = nc.declare_dram_parameter("ct_out", [D, rows], fo, isOutput=True)

    with tile.TileContext(nc) as tc:
        with (
            tc.tile_pool(name="const", bufs=1) as cst,
            tc.tile_pool(name="adj", bufs=adj_bufs) as apool,
            tc.tile_pool(name="b64", bufs=b64_bufs) as b64,
            tc.tile_pool(name="stg", bufs=stg_bufs) as stg,
            tc.tile_pool(name="gpsum", bufs=2, space="PSUM") as gpsum,
            tc.tile_pool(name="gatepsum", bufs=1, space="PSUM") as gatepsum,
        ):
            dma_engs = [nc.sync, nc.scalar]

            # SP ring: the two tiny consts PE needs first, then stripe 0.
            # ACT ring: hh chunks, then the tail-only consts.
            wxt_sb = cst.tile([D, 4 * D], f16)
            nc.sync.dma_start(wxt_sb[:], wxt[:])
            xt_sb = cst.tile([D, rows], f16)
            nc.sync.dma_start(xt_sb[:], xt[:])
            hh_sb = []
            for c in range(HHC):
                t = cst.tile([128, hkc * 2 * D], f16, tag=f"hh{c}")
                nc.scalar.dma_start(
                    t[:], hh[:, c * hkc * 2 * D:(c + 1) * hkc * 2 * D]
                )
                hh_sb.append(t)
            bias_sb = cst.tile([D, 4], f32)
            nc.scalar.dma_start(bias_sb[:], bias[:])
            wht_sb = cst.tile([D, 4 * D], f32)
            nc.scalar.dma_start(wht_sb[:], wht[:])
            ct_sb = cst.tile([D, rows], f32)
            nc.scalar.dma_start(ct_sb[:], ct[:])

            def body(_iv=None):
                # batched output stores of m-block i are deferred into
                # m-block i+1's stripe stream: by then they are computed,
                # so they never stall the HWDGE sequencers.
                pending = []

                for mbi in range(mb):
                    mbs = slice(mbi * mbw, (mbi + 1) * mbw)

                    # x-side gate matmuls: independent of the adjacency
                    # stream; fill PE early, leave only h-side on the tail.
                    pgs = []
                    for gi in range(4):
                        pg = gatepsum.tile([D, mbw], f32, tag=f"pg{gi}")
                        nc.tensor.matmul(
                            pg[:],
                            wxt_sb[:, gi * D:(gi + 1) * D],
                            xt_sb[:, mbs],
                            start=True,
                            stop=False,
                        )
                        pgs.append(pg)

                    gps = gpsum.tile([128, mbw], f32, tag="gps")
                    for g in range(gd):
                        first = (mbi == 0 and g == 0)
                        stripe = apool.tile([128, ktb * mbw], f16, tag="stripe")
                        rb = (mbi * gd + g) * 128
                        # every stripe is fetched in column chunks: PE gets
                        # work every ~chunk-DMA-time instead of idling a
                        # whole stripe DMA (keeps HAM from re-throttling),
                        # and boundary stripes start/finish incrementally.
                        cw = ktb * mbw // chunk_edges
                        ca = ktb // chunk_edges       # k-tiles per chunk
                        for ci in range(chunk_edges):
                            eng = dma_engs[0] if first else dma_engs[ci % 2]
                            eng.dma_start(
                                stripe[:, ci * cw:(ci + 1) * cw],
                                adjt[rb:rb + 128, ci * cw:(ci + 1) * cw],
                            )
                            if g == 1 and ci == 0 and pending:
                                for pi, (dst, tsb) in enumerate(pending):
                                    dma_engs[pi % 2].dma_start(dst, tsb[:])
                                pending.clear()
                            for a in range(ci * ca, (ci + 1) * ca):
                                kti = g * ktb + a
                                hc, ho = kti // hkc, kti % hkc
                                nc.tensor.matmul(
                                    gps[:],
                                    hh_sb[hc][:, ho * 2 * D:(ho + 1) * 2 * D],
                                    stripe[:, a * mbw:(a + 1) * mbw],
                                    start=(kti == 0),
                                    stop=(kti == kt - 1),
                                )
                    # tail in quarter-width slices so PE/ACT/DVE pipeline;
                    # quarters land in per-m-block staging tiles so each
                    # m-block issues just two batched output stores.
                    cts = stg.tile([D, mbw], fo, tag="cts")
                    hts = stg.tile([D, mbw], fo, tag="hts")
                    hw = mbw // tail_splits
                    for h in range(tail_splits):
                        hs = slice(h * hw, (h + 1) * hw)       # within m-block
                        hg = slice(mbi * mbw + h * hw, mbi * mbw + (h + 1) * hw)
                        gtb = b64.tile([D, hw], f32, tag=f"gtb{h}")
                        nc.vector.tensor_copy(gtb[:], gps[0:D, hs])
                        nc.vector.tensor_add(gtb[:], gtb[:], gps[D:2 * D, hs])
                        gates = []
                        for gi, fname in enumerate(_GATE_FUNCS):
                            pg = pgs[gi]
                            nc.tensor.matmul(
                                pg[:, hs],
                                wht_sb[:, gi * D:(gi + 1) * D],
                                gtb[:],
                                start=False,
                                stop=(h == tail_splits - 1),
                                skip_group_check=True,
                            )
                            gate_sb = b64.tile([D, hw], f32, tag=f"gate{gi}{h}")
                            nc.scalar.activation(
                                gate_sb[:],
                                pg[:, hs],
                                getattr(act, fname),
                                bias=bias_sb[:, gi:gi + 1],
                            )
                            gates.append(gate_sb)
                        it_, ft_, ot_, ut_ = gates

                        t1 = b64.tile([D, hw], f32, tag=f"t1{h}")
                        nc.vector.tensor_mul(t1[:], ft_[:], ct_sb[:, hg])
                        t2 = b64.tile([D, hw], f32, tag=f"t2{h}")
                        nc.vector.tensor_mul(t2[:], it_[:], ut_[:])
                        nc.vector.tensor_add(cts[:, hs], t1[:], t2[:])
                        tct = b64.tile([D, hw], f32, tag=f"tct{h}")
                        nc.scalar.activation(tct[:], cts[:, hs], act.Tanh)
                        nc.vector.tensor_mul(hts[:, hs], ot_[:], tct[:])
                    og = slice(mbi * mbw, (mbi + 1) * mbw)
                    if mbi == mb - 1:
                        # HWDGE rings are idle at the end: issue directly
                        nc.sync.dma_start(ct_out[:, og], cts[:])
                        nc.scalar.dma_start(ht_out[:, og], hts[:])
                    else:
                        pending.append((ct_out[:, og], cts))
                        pending.append((ht_out[:, og], hts))

            if repeat == 1:
                for _ in range(unroll):
                    body()
            else:
                # unroll>1 amortizes the per-trip InstAllEngineBarrier that
                # For_i inserts in its semaphore-reset block
                with tc.For_i(0, repeat, 1) as _i:
                    for _ in range(unroll):
                        body(_i)

    if split_waits:
        _split_excess_waits(nc)
    return nc


def make_in_maps(inputs, n=N, n_cores=N_CORES):
    """Host-side sharding + relayout. Returns per-core input dicts."""
    rows = n // n_cores
    kt = n // 128
    mb = rows // MBW
    gd = kt // KTB
    adj = np.asarray(inputs["adj_matrix"], dtype=np.float32)
    H = np.asarray(inputs["Ht_1"], dtype=np.float32)
    ht = np.asarray(inputs["ht"], dtype=np.float32)
    Ct_1 = np.asarray(inputs["Ct_1"], dtype=np.float32)

    # H = hi + lo to ~2^-22: the hi/lo fp16 pair is packed side by side
    # per k-tile ([128, kt*128]) so one matmul computes both products.
    Hh32 = H.astype(np.float16).astype(np.float32)
    packed = np.empty((128, kt, 2 * D), dtype=np.float16)
    packed[:, :, :D] = Hh32.reshape(kt, 128, D).transpose(1, 0, 2)
    packed[:, :, D:] = (H - Hh32).reshape(kt, 128, D).transpose(1, 0, 2)
    hh = np.ascontiguousarray(packed.reshape(128, kt * 2 * D))

    gate_w = ("Wxi", "Wxf", "Wxo", "Wxc")
    gate_h = ("Whi", "Whf", "Who", "Whc")
    wxt = np.concatenate(
        [np.asarray(inputs[g + "_w"], np.float32).T for g in gate_w], axis=1
    ).astype(np.float16)
    wht = np.concatenate(
        [np.asarray(inputs[g + "_w"], np.float32).T for g in gate_h], axis=1
    )
    # adj is shifted by -0.5 before the fp16 cast (halves quantization
    # error for uniform(0,1) entries). g = (adj-0.5)@H + 0.5*colsum(H)
    # broadcast over rows; the second term passes through the h-side
    # Linear as a per-feature constant, folded into the gate bias here.
    colsum = H.astype(np.float64).sum(axis=0)
    bias = np.stack(
        [
            np.asarray(inputs[gx + "_b"], np.float64)
            + np.asarray(inputs[gh + "_b"], np.float64)
            + 0.5 * (np.asarray(inputs[gh + "_w"], np.float64) @ colsum)
            for gx, gh in zip(gate_w, gate_h)
        ],
        axis=1,
    ).astype(np.float32)
    wxt = np.ascontiguousarray(wxt)
    wht = np.ascontiguousarray(wht)
    bias = np.ascontiguousarray(bias)

    in_maps = []
    for c in range(n_cores):
        rs = slice(c * rows, (c + 1) * rows)
        adjt_c = np.ascontiguousarray(adj[rs].T)
        adjt_c -= np.float32(0.5)
        a16 = adjt_c.astype(np.float16)          # [n, rows]
        # stripe-contiguous permute: [(mb gd) 128, ktb*mbw] where the row
        # block (mbi*gd+g)*128 holds k-rows (g*ktb .. )*128 interleaved as
        # [p, a, mj] -> flat [128, ktb*mbw] for m-cols mbi*mbw..+mbw.
        a5 = a16.reshape(gd, KTB, 128, mb, MBW).transpose(3, 0, 2, 1, 4)
        adjt_s = np.ascontiguousarray(a5.reshape(mb * gd * 128, KTB * MBW))
        in_maps.append(
            {
                "adjt": adjt_s,
                "hh": hh,
                "xt": np.ascontiguousarray(ht[rs].T).astype(np.float16),
                "ct": np.ascontiguousarray(Ct_1[rs].T),
                "wxt": wxt,
                "wht": wht,
                "bias": bias,
            }
        )
    return in_maps


def gather(results):
    Ht = np.concatenate(
        [np.asarray(r["ht_out"], np.float32).T for r in results], axis=0
    )
    Ct = np.concatenate(
        [np.asarray(r["ct_out"], np.float32).T for r in results], axis=0
    )
    return np.ascontiguousarray(Ht), np.ascontiguousarray(Ct)


_PROGRAM_CACHE = {}


def kernel(**inputs):
    from concourse.bass_utils import run_bass_kernel_spmd

    if "nc" not in _PROGRAM_CACHE:
        _PROGRAM_CACHE["nc"] = build()
    nc = _PROGRAM_CACHE["nc"]
    in_maps = make_in_maps(inputs)
    res = run_bass_kernel_spmd(nc, in_maps, list(range(N_CORES)))
    return gather(res.results)
